# revision 154
# baseline (speedup 1.0000x reference)
"""Trainium2 Bass kernel for a dense transformer encoder layer.

Problem: B=2, S=2048, D=1024, H=16 heads (W=64), F=4096, fp32 in/out.

Sharding: 8 cores = 2 batches x 4 sequence chunks of 512 tokens. Each core
computes K/V for its batch's full sequence and Q/attention/FFN for its own
512-token chunk. No collectives. Each core's xT8 is host-ROTATED so its
own query chunk sits at token offset 0: Q reads xT8[:, :, 0:SCH] directly
(no separate xq8 input - 0.5MB off the startup-critical DMA prefix);
softmax/attn-V are permutation-invariant over keys so the rotated K/V
order is exactly equivalent (mask bias rotated to match).

Precision plan (rel-err budget 2e-2, measured ~2e-3):
- K/V/Q projections, attn-V and the softmax normalizer Z run in fp8 e4m3
  with DoubleRow matmuls (0.5 cyc/row, 256-deep contraction). Attention's
  contribution to the residual stream is ~1% of its magnitude, so fp8
  noise there is suppressed ~100x.
- scores and out-proj run in fp8 without DoubleRow (score contraction is
  only 64 deep and out-proj wants 128 output rows; DR's M<=64 limit gives
  no win there).
- FFN fc1 runs in f32r (1 cyc/row like bf16 in the cost model, but full
  precision); fc2 in bf16. fp8 FFN was measured at ~2e-2 rel err - over
  budget - and hi/lo splits cost exactly what bf16 does.
- LayerNorm path stays fp32 (f32r matmuls for stats + rank-1 apply).

LN1 is never applied to the stream: g1 is folded into w1 host-side and
fc1 consumes the raw residual r1 directly; the mean/rstd become a
per-token affine on the fc1 evac (z = rstd*psg - (u*rstd)*c1 + bf1 +
b1@w1, with rank-1 tensors T_rstd/T_urstd built once per LN). The fc2
residual h1 = g1*(r1-u)*rstd + b1 is rebuilt from r1 with two DVE ops +
one scalar_tensor_tensor (b1 folded into bf2). This removes the LN1
apply from the out-proj->fc1 critical path (~12us) and drops the h1T
tile. LN2 is reassociated as oT = g2*rstd*(r2 - u) + b2 with the
mean-subtract on the IDLE PE: psd = I@r2 + (-ones)(x)u (identity
stationary copy + rank-1 accumulation into PSUM, both f32r matmuls
runnable right at stats-complete), leaving DVE just ONE stt per chunk
(g2 in the scalar slot, rstd via Pool partition_broadcast); +b2 and
the single f32->bf16 rounding ride the idle Act engine. This nearly
halves the serial DVE tail stream vs the 2-op/dc form (which itself
was the DVE-only ISA floor). fc1's T_rstd/T_urstd are likewise built
with Pool partition_broadcast (no PSUM tiles, so the psg ring isn't
serialized behind the rstd chain - worth ~6us). The output is bf16
(halves the out-DMA drain; host converts to f32).
NOTE: DVE must not read-modify-write PSUM in place
(NRT_EXEC_UNIT_UNRECOVERABLE on hw) - evacs go PSUM -> SBUF once, then
SBUF-only ops.

Layouts: activations transposed ([feature, token]); kT/qT live on SBUF
partitions 0-63 (DR outputs must start at partition 0 with M<=64), one
64-feature chunk per head, so scores for head h need only K/Q chunk h -
the exp stream starts ~5us into the kernel. V ([token, feature]) and hT
halves on partitions 64-127 are placed via SBUF->SBUF DMA. Z rides a
second DR matmul with a replicated-ones stationary (matmul cost depends
only on moving size, so 64-row replication is free) and normalization is
one DVE reciprocal + one multiply per head.

Schedule: Q upfront, then per head h: scores(h) kcp-pairs feeding exp(h)
on Act, with filler units (K-chunk(h+1), V tiles in the first 6 iterations,
and the 6-head-deferred attn-V/Z/normalize) interleaved between score
pairs so the PE never queues long dependent runs. Scores get a dedicated
2-deep PSUM ring; K/V/Q/attn-V share another; LN stats accumulate inside
the producing loops. Then out-proj, fc1-from-r1 (w1 f32r streamed over 3
DMA queues), fc2 + residual rebuild, fine-grained LN2 with inline output
DMA. Measured via TimelineSim: 312813 ns, rel err 3.5e-3.
NOTE: a DVE op cannot take TWO PSUM operands (BIR verifier rejects) -
T_rstd is a warm-PE rank-1 into ppk PSUM, evac'd to SBUF by idle Act
(faster than the Pool broadcast's q7 launch) so the stt reads one
PSUM (psd) + one SBUF (T_rstd) operand.

Act tables: every set holds Identity/Square but Sqrt and Gelu live in
different sets, so warm ONLY Sqrt after attention (load hides under
out-proj) and again after the last gelu (hides under fc2); fc1's first
gelu takes its own load off-critical. Warming Gelu+Sqrt together (old
scheme) thrashed ~5 table loads. A dep-free warm gets HOISTED by the
tile scheduler next to the first warm - anchor it by reading the last
gelu's output.

Tried and rejected: fp8 FFN (2.3e-2 err), DoubleRow out-proj (evac
doubling eats the PE saving), fc2 token-halving (re-streaming w2 8MB
contends the shared DMA device, +43us), wq-first / Q-early DMA orders
(start is bound by the SUM of critical-set bytes wq+xq+wk+xT on the
shared DMA device; Q-early just fragments PE idle and resets pstate),
gpsimd tensor_tensor for LN2 applies (BIR verifier rejects), DVE x*x
self-multiply (verifier rejects duplicate operands - use Act Square),
Act Rsqrt (bass accuracy guard), hv-precompute of the fc2 residual
(+1.4us, no win), DEFER=5/7 (worse/SBUF-overflow), SBUF-staged LN2
applies hoping for the 2x DVE mode (+3us, mode not granted), pairing
LN2 applies (+0.6us, first DMA waits the pair), interleaving drain
attn-units with early out-proj chains (+12us full / +0.4us gentle
1-blocked-mm variant - the drain idle is not queue-recoverable),
3-queue out-DMA rotation (+0.1us), u-row copy on Act (+0.6us - queues
behind the last sq2/var Squares), last-LN2-chunk halving (+0.3us).
"""
import numpy as np
import ml_dtypes
import concourse.bass as bass
from concourse import bacc
import concourse.mybir as mybir
import concourse.tile as tile
from concourse.bass import ts
from concourse.bass_utils import run_bass_kernel_spmd

P = 128
B, S, D, H, W, F = 2, 2048, 1024, 16, 64, 4096
DC = D // P            # 8 128-feature chunks
FC = F // P            # 32
FCH = D // W           # 16 64-feature chunks (one per head)
TC = S // P            # 16 key-token chunks
SCH = 512              # tokens per core
EPS = 1e-12
SCALE = 1.0 / np.sqrt(np.float32(W))
DEFER = 6              # attn-V lags scores by this many heads
NPROBS = DEFER + 1

F32 = mybir.dt.float32
F32R = mybir.dt.float32r
FP8 = mybir.dt.float8e4
BF16 = mybir.dt.bfloat16
DR = mybir.MatmulPerfMode.DoubleRow

_cache = {}


def _ln_rstd(nc, ps_u, ps_v, pool, ru2_row, tag, w=SCH):
    """Stats -> rstd chain: rstd = 1/sqrt(ps_v - ps_u^2 + eps); also fills
    ru2_row (a [1, w] AP) with u*rstd. Returns the [1, w] f32r rstd tile.
    ps_u/ps_v are [1, w] APs of PSUM stats."""
    at = mybir.ActivationFunctionType
    var = pool.tile([1, w], F32, tag=tag + "var", name="var")
    rstd = pool.tile([1, w], F32R, tag=tag + "rstd", name="rstd")
    sd = pool.tile([1, w], F32, tag=tag + "sd", name="sd")
    nc.scalar.activation(var[:], ps_u, at.Square)
    nc.vector.tensor_tensor(var[:], ps_v, var[:], mybir.AluOpType.subtract)
    nc.scalar.activation(sd[:], var[:], at.Sqrt, bias=EPS)
    nc.vector.reciprocal(rstd[:], sd[:])
    if ru2_row is not None:
        nc.vector.tensor_tensor(ru2_row, ps_u, rstd[:],
                                mybir.AluOpType.mult)
    return rstd


def _build(masked=False):
    at = mybir.ActivationFunctionType
    nc = bacc.Bacc("TRN2", target_bir_lowering=False)

    xT_d = nc.dram_tensor("xT8", [P, DC, S], FP8, kind="ExternalInput")
    xs_d = nc.dram_tensor("xs", [P, DC, SCH], F32R, kind="ExternalInput")
    wq_d = nc.dram_tensor("wq8", [P, DC, D], FP8, kind="ExternalInput")
    wk_d = nc.dram_tensor("wk8", [P, DC, D], FP8, kind="ExternalInput")
    wv_d = nc.dram_tensor("wv8", [P, DC, D], FP8, kind="ExternalInput")
    wo_d = nc.dram_tensor("wo8", [P, DC, D], FP8, kind="ExternalInput")
    w1_d = nc.dram_tensor("w1r", [P, DC, F], F32R, kind="ExternalInput")
    w2_d = nc.dram_tensor("w2b", [P, FC, D], BF16, kind="ExternalInput")
    c1n_d = nc.dram_tensor("c1n", [P, FC], F32, kind="ExternalInput")
    bq_d = nc.dram_tensor("bq64", [W, FCH], F32, kind="ExternalInput")
    bk_d = nc.dram_tensor("bk64", [W, FCH], F32, kind="ExternalInput")
    bv_d = nc.dram_tensor("bvr", [1, D], F32R, kind="ExternalInput")
    bo_d = nc.dram_tensor("bo", [P, DC], F32, kind="ExternalInput")
    bf1_d = nc.dram_tensor("bf1", [P, FC], F32, kind="ExternalInput")
    bf2_d = nc.dram_tensor("bf2", [P, DC], F32, kind="ExternalInput")
    gnb1_d = nc.dram_tensor("gnb1", [2, D], F32R, kind="ExternalInput")
    gnb2_d = nc.dram_tensor("gnb2", [2, D], F32R, kind="ExternalInput")
    g1c_d = nc.dram_tensor("g1c", [P, DC], F32, kind="ExternalInput")
    g2c_d = nc.dram_tensor("g2c", [P, DC], F32, kind="ExternalInput")
    b2c_d = nc.dram_tensor("b2c", [P, DC], F32, kind="ExternalInput")
    invd_d = nc.dram_tensor("invd", [P, 1], F32R, kind="ExternalInput")
    idn_d = nc.dram_tensor("ident", [P, P], F32R, kind="ExternalInput")
    ones5_d = nc.dram_tensor("ones512", [1, SCH], F32R, kind="ExternalInput")
    mb_d = nc.dram_tensor("mb", [P, TC], F32, kind="ExternalInput") if masked else None
    # bf16 output: halves the tail's out-DMA drain; host converts to f32.
    # Adds <=0.4% rounding on the largest elements - 4x under the err gate.
    out_d = nc.dram_tensor("outT", [P, DC, SCH], BF16, kind="ExternalOutput")

    # V tiles (tcl, grp) packed into the first head-iterations
    v_tiles = [(tcl, g) for tcl in range(TC) for g in range(2)]
    v_sched = {0: v_tiles[0:6], 1: v_tiles[6:12], 2: v_tiles[12:17],
               3: v_tiles[17:22], 4: v_tiles[22:27], 5: v_tiles[27:32]}

    with nc.allow_low_precision(reason="fp8/bf16 by design"), \
         tile.TileContext(nc) as tc:
        with tc.tile_pool(name="small", bufs=1) as small, \
             tc.tile_pool(name="pps", bufs=2, space="PSUM") as pps, \
             tc.tile_pool(name="ppk", bufs=2, space="PSUM") as ppk:
            # ---- long-lived tiles, reverse order of death ----
            hT, hT_free = tc.tile([P, DC, SCH], FP8, name="hT")
            wo8, wo8_free = tc.tile([P, DC, D], FP8, name="wo8")
            xs2, xs2_free = tc.tile([P, DC, SCH], F32R, name="xs2")
            kT, kT_free = tc.tile([W, FCH, S], FP8, name="kT")
            qT, qT_free = tc.tile([W, FCH, SCH], FP8, name="qT")
            vA, vA_free = tc.tile([P, TC, D], FP8, name="vA")

            ones8 = small.tile([P, 2, W], FP8)
            onesr = small.tile([1, P], F32R)
            bvb = small.tile([P, D], F32)
            bk_sb = small.tile([W, FCH], F32)
            bq_sb = small.tile([W, FCH], F32)
            bo_sb = small.tile([P, DC], F32)
            bf1_sb = small.tile([P, FC], F32)
            bf2_sb = small.tile([P, DC], F32)
            c1n_sb = small.tile([P, FC], F32)
            g1c_sb = small.tile([P, DC], F32)
            g2c_sb = small.tile([P, DC], F32)
            b2c_sb = small.tile([P, DC], F32)
            TT = small.tile([P, 3, SCH], F32R)
            invd = small.tile([P, 1], F32R)
            gnb1 = small.tile([2, D], F32R)
            ru2 = small.tile([2, SCH], F32R)
            epsc = small.tile([P, 1], F32)
            mb_sb = small.tile([P, TC], F32) if masked else None

            nc.vector.memset(epsc[:], EPS)
            nc.const_aps.aps[(F32, EPS)] = epsc[:]
            nc.vector.memset(ones8[:], 1.0)
            nc.sync.dma_start(onesr[:], ones5_d[:, 0:P])
            nc.sync.dma_start(ru2[1:2], ones5_d[:])

            with tc.tile_pool(name="wpool", bufs=1) as wpool:
                wk8 = wpool.tile([P, DC, D], FP8)
                wv8 = wpool.tile([P, DC, D], FP8)
                xT8 = wpool.tile([P, DC, S], FP8)

                horder = []
                for hp_ in range(H // 2):
                    horder.extend([2 * hp_ + 1, 2 * hp_])

                def k_half(hh, th, on_act=False):
                    # kT[:, hh, 1024*th:+1024]: g-outer so the wk8
                    # stationary is reused across both sub-regions
                    psk = ppk.tile([W, 1024], F32, tag="pk", name="psk")
                    for g in range(4):
                        for sub in range(2):
                            tok = 1024 * th + 512 * sub
                            nc.tensor.matmul(
                                psk[:, ts(sub, 512)],
                                wk8[:, 2 * g:2 * g + 2, ts(hh, W)],
                                xT8[:, 2 * g:2 * g + 2, tok:tok + 512],
                                start=(g == 0), stop=(g == 3),
                                perf_mode=DR)
                    if on_act:
                        nc.scalar.activation(kT[:, hh, ts(th, 1024)],
                                             psk[:], at.Identity,
                                             bias=bk_sb[:, hh:hh + 1])
                    else:
                        nc.vector.tensor_scalar(
                            kT[:, hh, ts(th, 1024)], psk[:],
                            bk_sb[:, hh:hh + 1], None,
                            mybir.AluOpType.add)

                # ---- Q projection upfront + bvb = ones (x) bv ----
                with tc.tile_pool(name="wqpool", bufs=1) as wqpool:
                    wq8 = wqpool.tile([P, DC, D], FP8)
                    bv_row = wqpool.tile([1, D], F32R)
                    # xT8 is host-ROTATED so this core's 512 query tokens
                    # sit at offset 0: Q reads xT8[:, :, 0:SCH] directly
                    # (softmax/attnV are permutation-invariant over keys),
                    # deleting the separate 0.5MB xq8 from the critical DMA
                    # prefix; wq8's second half is demoted past wv8 too.
                    nc.sync.dma_start(wq8[:, :, 0:512], wq_d[:, :, 0:512])
                    nc.gpsimd.dma_start(xT8[:, :, 0:512], xT_d[:, :, 0:512])
                    nc.scalar.dma_start(bq_sb[:], bq_d[:])
                    nc.scalar.dma_start(bv_row[:], bv_d[:])
                    nc.sync.dma_start(wq8[:, :, 512:], wq_d[:, :, 512:])
                    nc.scalar.dma_start(wk8[:], wk_d[:])
                    nc.gpsimd.dma_start(xT8[:, :, 512:1024],
                                        xT_d[:, :, 512:1024])
                    nc.gpsimd.dma_start(xT8[:, :, 1024:], xT_d[:, :, 1024:])
                    nc.scalar.dma_start(bk_sb[:], bk_d[:])
                    # wv8 halved: the first half races ahead of xT8's last
                    # chunk (v_tile's g=0,1 matmuls need only dc 0-3), the
                    # second follows - keeps K-path bytes flowing
                    nc.sync.dma_start(wv8[:, 0:4], wv_d[:, 0:4])
                    nc.scalar.dma_start(wv8[:, 4:], wv_d[:, 4:])
                    nc.sync.dma_start(bo_sb[:], bo_d[:])
                    nc.sync.dma_start(invd[:], invd_d[:])
                    if masked:
                        nc.sync.dma_start(mb_sb[:], mb_d[:])
                    # warm the Exp table while DMAs land
                    wrm = wqpool.tile([1, 1], F32)
                    nc.scalar.activation(wrm[:], epsc[0:1, :], at.Exp)
                    for hq in range(FCH):
                        psq = ppk.tile([W, 1024], F32, tag="pk", name="psq")[:, 0:SCH]
                        for g in range(4):
                            nc.tensor.matmul(psq[:],
                                             wq8[:, 2 * g:2 * g + 2, ts(hq, W)],
                                             xT8[:, 2 * g:2 * g + 2, 0:SCH],
                                             start=(g == 0), stop=(g == 3),
                                             perf_mode=DR)
                        nc.vector.tensor_scalar(qT[:, hq], psq[:],
                                                bq_sb[:, hq:hq + 1], None,
                                                mybir.AluOpType.add)
                        if hq == 7:
                            # first head's K lower half only: scores kc0-7
                            # need just tokens 0-1023, so the exp stream
                            # starts before xT8's last chunk lands; the
                            # upper half becomes a head-iter-0 filler
                            k_half(horder[0], 0, on_act=True)
                    psbv = pps.tile([P, D], F32, tag="ps", name="psbv")
                    nc.tensor.matmul(psbv[:, 0:512], onesr[:],
                                     bv_row[:, 0:512], start=True, stop=True)
                    nc.tensor.matmul(psbv[:, 512:], onesr[:],
                                     bv_row[:, 512:], start=True, stop=True)
                    nc.vector.tensor_copy(bvb[:], psbv[:])

                # out-proj/FFN-phase data streams during attention - keeps
                # 3MB off the DMA device in the startup-critical window
                nc.sync.dma_start(wo8[:], wo_d[:])
                for dcw in range(DC):
                    nc.gpsimd.dma_start(xs2[:, dcw], xs_d[:, dcw])
                nc.sync.dma_start(bf1_sb[:], bf1_d[:])
                nc.sync.dma_start(bf2_sb[:], bf2_d[:])
                nc.sync.dma_start(c1n_sb[:], c1n_d[:])
                nc.sync.dma_start(gnb1[:], gnb2_d[:])
                nc.sync.dma_start(g1c_sb[:], g1c_d[:])
                nc.sync.dma_start(g2c_sb[:], g2c_d[:])
                nc.sync.dma_start(b2c_sb[:], b2c_d[:])

                # ===== fused K/Q/scores/exp/V + deferred attn-V =====
                probs_tiles = {}
                with tc.tile_pool(name="prp", bufs=NPROBS) as prp, \
                     tc.tile_pool(name="stage", bufs=2) as stp:

                    def attn_units(hh):
                        """attn-V + Z + normalize for head hh (vA complete),
                        as schedulable units. pso/psz share one 2-bank slot,
                        allocated lazily at first use so the ring slot is not
                        claimed before other pool users emitted in between."""
                        pr = probs_tiles.pop(hh)
                        st_ = {}

                        def get_psoz():
                            if "t" not in st_:
                                st_["t"] = ppk.tile([W, D], F32, tag="pk",
                                                    name="psoz")
                            return st_["t"]

                        def mk_av(k0, k1, st, sp):
                            def u():
                                pso = get_psoz()[:, 0:SCH]
                                for kcp in range(k0, k1):
                                    nc.tensor.matmul(
                                        pso, vA[:, 2 * kcp:2 * kcp + 2,
                                                ts(hh, W)],
                                        pr[:, 2 * kcp:2 * kcp + 2, :],
                                        start=(st and kcp == k0),
                                        stop=(sp and kcp == k1 - 1),
                                        perf_mode=DR)
                            return u

                        def mk_z():
                            def u():
                                psz = get_psoz()[:, SCH:]
                                for kcp in range(TC // 2):
                                    nc.tensor.matmul(
                                        psz, ones8[:],
                                        pr[:, 2 * kcp:2 * kcp + 2, :],
                                        start=(kcp == 0),
                                        stop=(kcp == TC // 2 - 1),
                                        perf_mode=DR)
                            return u

                        def norm():
                            psoz = get_psoz()
                            pso, psz = psoz[:, 0:SCH], psoz[:, SCH:]
                            rzb = stp.tile([W, SCH], BF16, tag="rz", name="rzb")
                            nc.vector.reciprocal(rzb[:], psz)
                            hc, hp = hh // 2, W * (hh % 2)
                            if hp == 0:
                                nc.vector.tensor_tensor(hT[0:W, hc], pso,
                                                        rzb[:],
                                                        mybir.AluOpType.mult)
                            else:
                                hst = stp.tile([W, SCH], FP8, tag="hst",
                                               bufs=1, name="hst")
                                nc.vector.tensor_tensor(hst[:], pso, rzb[:],
                                                        mybir.AluOpType.mult)
                                nc.gpsimd.dma_start(hT[hp:hp + W, hc], hst[:])
                        return [mk_av(0, 4, True, False),
                                mk_av(4, 8, False, True), mk_z(), norm]

                    def v_tile(tcl, grp):
                        psv = ppk.tile([W, D], F32, tag="pk", name="psv")
                        t0 = tcl * P + grp * W
                        for g in range(4):
                            for half in range(2):
                                nc.tensor.matmul(
                                    psv[:, ts(half, 512)],
                                    xT8[:, 2 * g:2 * g + 2, t0:t0 + W],
                                    wv8[:, 2 * g:2 * g + 2, ts(half, 512)],
                                    start=(g == 0), stop=(g == 3),
                                    perf_mode=DR)
                        if grp == 0:
                            nc.vector.tensor_tensor(vA[0:W, tcl, :], psv[:],
                                                    bvb[0:W, :],
                                                    mybir.AluOpType.add)
                        else:
                            vst = stp.tile([W, D], FP8, tag="vst", name="vst")
                            nc.vector.tensor_tensor(vst[:], psv[:],
                                                    bvb[0:W, :],
                                                    mybir.AluOpType.add)
                            nc.gpsimd.dma_start(vA[W:P, tcl, :], vst[:])

                    for hi in range(H):
                        h = horder[hi]
                        # filler units to slot between scores/exp pairs
                        fillers = []
                        if hi == 0:
                            fillers.append(
                                lambda: k_half(horder[0], 1))
                        if hi + 1 < H:
                            hn = horder[hi + 1]
                            fillers.append(lambda a=hn: k_half(a, 0))
                            fillers.append(lambda a=hn: k_half(a, 1))
                        for (tcl, grp) in v_sched.get(hi, []):
                            fillers.append(
                                lambda a=tcl, b=grp: v_tile(a, b))
                        if hi >= DEFER:
                            fillers.extend(attn_units(horder[hi - DEFER]))
                        # scores + exp for head h, fillers interleaved
                        pr = prp.tile([P, TC, SCH], FP8, tag="probs", name="pr")
                        probs_tiles[h] = pr
                        for kcp in range(TC // 2):
                            pss = pps.tile([P, 2, 512], F32, tag="ps",
                                           name="pss")
                            for j in range(2):
                                kc = 2 * kcp + j
                                nc.tensor.matmul(pss[:, j], kT[:, h, ts(kc, P)],
                                                 qT[:, h], start=True, stop=True)
                            if masked:
                                for j in range(2):
                                    kc = 2 * kcp + j
                                    nc.scalar.activation(
                                        pr[:, kc, :], pss[:, j], at.Exp,
                                        bias=mb_sb[:, kc:kc + 1],
                                        scale=float(SCALE))
                            else:
                                nc.scalar.activation(
                                    pr[:, 2 * kcp:2 * kcp + 2, :], pss[:],
                                    at.Exp, scale=float(SCALE))
                            while fillers and len(fillers) >= TC // 2 - kcp:
                                fillers.pop(0)()
                        for f in fillers:
                            f()
                    for hi in range(H - DEFER, H):
                        for f in attn_units(horder[hi]):
                            f()
            vA_free()
            qT_free()
            kT_free()

            # ================= out-proj + residual + LN1 =================
            r1T, r1T_free = tc.tile([P, DC, SCH], F32R, name="r1T")
            sq1, sq1_free = tc.tile([P, DC, SCH], F32R, name="sq1")
            # identity stationary for LN2's PE-side mean-subtract; lives
            # post-attention only (small pool had no room in-attention)
            idn, idn_free = tc.tile([P, P], F32R, name="idn")
            nc.sync.dma_start(idn[:], idn_d[:])
            # warm ONLY Sqrt here (table load hides under out-proj PE work):
            # sqrt_and_others also holds Identity/Square, so rstd1's Sqrt
            # runs load-free; fc1's first Gelu then takes its one load
            # off-critical. Warming Gelu here (as before) double-thrashed.
            wrm2 = small.tile([1, 4], F32)
            nc.scalar.activation(wrm2[:, 1:2], epsc[0:1, :], at.Sqrt)
            ps_u1 = ppk.tile([1, SCH], F32, tag="pk", name="ps_u1")
            ps_v1 = ppk.tile([1, SCH], F32, tag="pk", name="ps_v1")
            g1T, g1T_free = tc.tile([P, FC, SCH], BF16, name="g1T")
            qs = [nc.sync, nc.scalar, nc.gpsimd]
            pf1_ctx = tc.tile_pool(name="pf1", bufs=3)
            pf1 = pf1_ctx.__enter__()
            # prefetch the first w1 tiles under out-proj so the fc1 matmuls
            # can fill the PE gap while the rstd1 chain runs
            w1pre = []
            for fcp in range(3):
                w1t = pf1.tile([P, DC, 2 * P], F32R, tag="w1t", name="w1t")
                qs[fcp % 3].dma_start(w1t[:], w1_d[:, :, ts(fcp, 2 * P)])
                w1pre.append(w1t)
            for dp in range(DC):
                psr = pps.tile([P, SCH], F32, tag="ps", name="psr")
                for dc in range(DC):
                    nc.tensor.matmul(psr[:], wo8[:, dc, ts(dp, P)],
                                     hT[:, dc],
                                     start=(dc == 0), stop=(dc == DC - 1))
                # evac + square on DVE: Act still has an exp backlog here,
                # and the stats chain below gates the whole FFN
                nc.vector.scalar_tensor_tensor(
                    r1T[:, dp], psr[:], bo_sb[:, dp:dp + 1], xs2[:, dp],
                    mybir.AluOpType.add, mybir.AluOpType.add)
                nc.scalar.activation(sq1[:, dp], r1T[:, dp], at.Square)
                nc.tensor.matmul(ps_u1[:], invd[:], r1T[:, dp],
                                 start=(dp == 0), stop=(dp == DC - 1))
                nc.tensor.matmul(ps_v1[:], invd[:], sq1[:, dp],
                                 start=(dp == 0), stop=(dp == DC - 1))
            # ================= FFN =================
            # fc1 reads r1T directly: g1 is folded into w1 (host side) and
            # the LN1 mean/rstd become a per-token affine applied AFTER the
            # matmul: z = rstd*psg - (u*rstd)*c1 + (bf1 + b1@w1). This
            # unblocks fc1 from the LN1 apply. h1T (needed only for the fc2
            # residual) is produced concurrently below.
            rstd1 = _ln_rstd(nc, ps_u1[:], ps_v1[:], small, ru2[0:1], "ln1")
            # rank-1 tensors via Pool partition-broadcast: no PSUM tiles, so
            # the psg ring isn't serialized behind the rstd chain and the
            # first fc1 chains fill the PE gap
            nc.gpsimd.partition_broadcast(TT[:, 0], rstd1[:])
            nc.gpsimd.partition_broadcast(TT[:, 1], rstd1[:])
            nc.gpsimd.partition_broadcast(TT[:, 2], ru2[0:1])
            with tc.tile_pool(name="gst", bufs=2) as gsp:
                for fcp in range(FC // 2):
                    if fcp < 3:
                        w1t = w1pre[fcp]
                    else:
                        w1t = pf1.tile([P, DC, 2 * P], F32R, tag="w1t",
                                       name="w1t")
                        qs[fcp % 3].dma_start(w1t[:],
                                              w1_d[:, :, ts(fcp, 2 * P)])
                    psg = pps.tile([P, 2, SCH], F32, tag="ps", name="psg")
                    for j in range(2):
                        for dc in range(DC):
                            nc.tensor.matmul(psg[:, j], w1t[:, dc, ts(j, P)],
                                             r1T[:, dc],
                                             start=(dc == 0),
                                             stop=(dc == DC - 1))
                    gst = gsp.tile([P, 2, SCH], F32, tag="gst", name="gst")
                    nc.vector.tensor_tensor(gst[:], psg[:], TT[:, 0:2],
                                            mybir.AluOpType.mult)
                    for j in range(2):
                        fc = 2 * fcp + j
                        nc.vector.scalar_tensor_tensor(
                            gst[:, j], TT[:, 2], c1n_sb[:, fc:fc + 1],
                            gst[:, j], mybir.AluOpType.mult,
                            mybir.AluOpType.add)
                        nc.scalar.activation(g1T[:, fc], gst[:, j], at.Gelu,
                                             bias=bf1_sb[:, fc:fc + 1])
            pf1_ctx.__exit__(None, None, None)
            # swap back to the sqrt set after the last gelu: fc2's
            # Identity/Square live there too, so LN2's Sqrt is load-free.
            # The input MUST be the last gelu's output - a dep-free warm
            # gets hoisted by the scheduler to before fc1's gelus, which
            # then re-load the gelu set (NaN from sqrt(neg) is unread).
            nc.scalar.activation(wrm2[:, 2:3], g1T[0:1, FC - 1, 0:1],
                                 at.Sqrt)
            r2T, r2T_free = tc.tile([P, DC, SCH], F32R, name="r2T")
            sq2, sq2_free = tc.tile([P, DC, SCH], F32R, name="sq2")
            ps_u2 = ppk.tile([1, SCH], F32, tag="pk", name="ps_u2")
            ps_v2 = ppk.tile([1, SCH], F32, tag="pk", name="ps_v2")
            with tc.tile_pool(name="pw2", bufs=2) as pw2:
                for dp in range(DC):
                    w2t = pw2.tile([P, FC, P], BF16, tag="w2t", name="w2t")
                    nc.sync.dma_start(w2t[:, 0:FC // 2],
                                      w2_d[:, 0:FC // 2, ts(dp, P)])
                    nc.scalar.dma_start(w2t[:, FC // 2:],
                                        w2_d[:, FC // 2:, ts(dp, P)])
                    psf = pps.tile([P, SCH], F32, tag="ps", name="psf")
                    for fc in range(FC):
                        nc.tensor.matmul(psf[:], w2t[:, fc], g1T[:, fc],
                                         start=(fc == 0), stop=(fc == FC - 1))
                    nc.scalar.activation(r2T[:, dp], psf[:], at.Identity,
                                          bias=bf2_sb[:, dp:dp + 1])
                    # residual h1 = g1*(r1 - u)*rstd + b1 rebuilt from r1T
                    # (b1 is folded into bf2 host-side); bvb is a scratch
                    # tile here - in-order DVE makes one buffer race-free
                    nc.vector.tensor_tensor(bvb[:, 0:SCH], r1T[:, dp],
                                            TT[:, 0], mybir.AluOpType.mult)
                    nc.vector.tensor_tensor(bvb[:, 0:SCH], bvb[:, 0:SCH],
                                            TT[:, 2],
                                            mybir.AluOpType.subtract)
                    nc.vector.scalar_tensor_tensor(
                        r2T[:, dp], bvb[:, 0:SCH], g1c_sb[:, dp:dp + 1],
                        r2T[:, dp], mybir.AluOpType.mult,
                        mybir.AluOpType.add)
                    nc.scalar.activation(sq2[:, dp], r2T[:, dp], at.Square)
                    nc.tensor.matmul(ps_u2[:], invd[:], r2T[:, dp],
                                     start=(dp == 0), stop=(dp == DC - 1))
                    nc.tensor.matmul(ps_v2[:], invd[:], sq2[:, dp],
                                     start=(dp == 0), stop=(dp == DC - 1))

            # ================= LN2 + out =================
            # oT = g2*rstd*(r2 - u) + b2, reassociated so the (r2 - T_u)
            # subtractions depend only on the mean u - they start right at
            # stats-complete and overlap the whole Sqrt/reciprocal chain.
            # g2 rides the stt scalar slot, rstd a Pool broadcast, +b2 the
            # idle Act engine. No PE rank-1s, no PSUM in the applies.
            # the mean-subtract moves to the idle PE: psd = I@r2 - ones(x)u
            # (identity stationary + rank-1 accumulation), so DVE does only
            # ONE stt per chunk - the serial tail stream nearly halves
            oT, oT_free = tc.tile([P, DC, SCH], BF16, name="oT")
            dT, dT_free = tc.tile([P, DC, SCH], F32, name="dT")
            u_ng = small.tile([1, SCH], F32R, tag="ln2u", name="u_ng")
            nc.vector.tensor_scalar(u_ng[:], ps_u2[:], -1.0, None,
                                    mybir.AluOpType.mult)
            rstd2 = _ln_rstd(nc, ps_u2[:], ps_v2[:], small, None, "ln2")
            # T_rstd via a warm-PE rank-1 into a ppk PSUM slot instead of a
            # Pool broadcast (~1.2us launch+sem) - stt's in1 reads PSUM at
            # no extra init since psd is already a PSUM operand
            psT2 = ppk.tile([P, SCH], F32, tag="pk", name="psT2")
            nc.tensor.matmul(psT2[:], onesr[:], rstd2[:], start=True,
                             stop=True)
            # evac via idle Act: DVE can't take two PSUM operands (psd is
            # already PSUM), and Act beats the Pool broadcast's q7 launch
            nc.scalar.activation(TT[:, 0], psT2[:], at.Identity)
            for dc in range(DC):
                psd = pps.tile([P, SCH], F32, tag="ps", name="psd")
                nc.tensor.matmul(psd[:], idn[:], r2T[:, dc], start=True,
                                 stop=False)
                nc.tensor.matmul(psd[:], onesr[:], u_ng[:], start=False,
                                 stop=True)
                nc.vector.scalar_tensor_tensor(
                    dT[:, dc], psd[:], g2c_sb[:, dc:dc + 1],
                    TT[:, 0], mybir.AluOpType.mult, mybir.AluOpType.mult)
                nc.scalar.activation(oT[:, dc], dT[:, dc], at.Identity,
                                     bias=b2c_sb[:, dc:dc + 1])
                eng = nc.sync if dc % 2 == 0 else nc.scalar
                eng.dma_start(out_d[:, dc], oT[:, dc])
            dT_free()
            oT_free()
            sq2_free()
            r2T_free()
            g1T_free()
            idn_free()
            sq1_free()
            r1T_free()
            xs2_free()
            wo8_free()
            hT_free()

    nc.compile()
    return nc


def kernel(**inputs):
    x = np.asarray(inputs["x"], dtype=np.float32)
    mask = np.asarray(inputs["mask"])
    f = {k: np.asarray(inputs[k], dtype=np.float32) for k in
         ["wq", "bq", "wk", "bk", "wv", "bv", "wo", "bo", "g1", "b1",
          "w1", "bf1", "w2", "bf2", "g2", "b2"]}

    masked = not bool(np.all(mask == 1))
    key = ("nc", masked)
    if key not in _cache:
        _cache[key] = _build(masked)
    nc = _cache[key]
    _cache["nc"] = nc  # test.py reads this for TimelineSim

    def wlay(w, pc):  # [K, M] -> [P, K//P, M]
        return np.ascontiguousarray(w.reshape(pc, P, w.shape[1]).transpose(1, 0, 2))

    def blay(b):      # [M] -> [P, M//P]
        return np.ascontiguousarray(b.reshape(-1, P).T)

    # fold g1 into w1; LN1 mean/rstd applied post-matmul inside the kernel:
    # z = rstd*(r1 @ (g1*w1)) - (u*rstd)*c1 + (bf1 + b1 @ w1)
    w1f = f["w1"].astype(np.float64)
    w1g = w1f * f["g1"].astype(np.float64)[:, None]
    c1 = w1g.sum(0)
    d1 = (f["b1"].astype(np.float64)[:, None] * w1f).sum(0)

    fp8 = ml_dtypes.float8_e4m3fn
    bf16 = ml_dtypes.bfloat16
    shared = {
        "wq8": wlay(f["wq"], DC).astype(fp8),
        "wk8": wlay(f["wk"], DC).astype(fp8),
        "wv8": wlay(f["wv"], DC).astype(fp8),
        "wo8": wlay(f["wo"], DC).astype(fp8),
        "w1r": wlay(w1g.astype(np.float32), DC),
        "w2b": wlay(f["w2"], FC).astype(bf16),
        "c1n": blay((-c1).astype(np.float32)),
        "invd": np.full((P, 1), 1.0 / D, np.float32),
        "ones512": np.ones((1, SCH), np.float32),
        "gnb1": np.ascontiguousarray(
            np.stack([f["g1"], -f["b1"]]).astype(np.float32)),
        "gnb2": np.ascontiguousarray(
            np.stack([f["g2"], -f["b2"]]).astype(np.float32)),
        "bq64": np.ascontiguousarray(f["bq"].reshape(FCH, W).T),
        "bk64": np.ascontiguousarray(f["bk"].reshape(FCH, W).T),
        "bvr": f["bv"].reshape(1, D),
        "bo": blay(f["bo"]),
        "bf1": blay((f["bf1"].astype(np.float64) + d1).astype(np.float32)),
        "bf2": blay(f["bf2"] + f["b1"]),
        "g1c": blay(f["g1"]),
        "g2c": blay(f["g2"]),
        "b2c": blay(f["b2"]),
        "ident": np.eye(P, dtype=np.float32),
    }

    in_maps = []
    for c in range(8):
        b, sq = c // 4, c % 4
        xTb = np.ascontiguousarray(x[b].T.reshape(DC, P, S).transpose(1, 0, 2))
        m = dict(shared)
        # rotate tokens so this core's 512-query chunk is at offset 0;
        # attention is permutation-invariant over keys, so K/V/probs in
        # rotated order give identical outputs (mask bias rotated to match)
        xrot = np.roll(xTb, -sq * SCH, axis=2)
        m["xT8"] = np.ascontiguousarray(xrot).astype(fp8)
        m["xs"] = np.ascontiguousarray(xrot[:, :, 0:SCH])
        if masked:
            mbias = (-10000.0 * (1.0 - mask[b].astype(np.float32)))
            mbias = np.roll(mbias, -sq * SCH)
            m["mb"] = np.ascontiguousarray(mbias.reshape(TC, P).T)
        in_maps.append(m)

    res = run_bass_kernel_spmd(nc, in_maps, core_ids=list(range(8)))
    _cache["last_res"] = res

    out = np.empty((B, S, D), np.float32)
    for c in range(8):
        b, sq = c // 4, c % 4
        oT = res.results[c]["outT"].astype(np.float32)  # [P, DC, SCH] bf16
        out[b, sq * SCH:(sq + 1) * SCH, :] = oT.transpose(2, 1, 0).reshape(SCH, D)
    return out

